# revision 22
# baseline (speedup 1.0000x reference)
"""BRITS bidirectional RNN imputation kernel for Trainium2 (Bass/Tile, 8 NeuronCores).

Sharding: cores 0-3 run the forward RITS on batch quarters 0-3; cores 4-7 run the
backward RITS (time-reversed inputs) on batch quarters 0-3. The time recurrence is
local per core. Host averages the two direction outputs (0.5*(f + b)) at the end.

Per-core layout is feature-on-partition, batch-on-free ("transposed"): activations
are [F or H chunk (partitions), batch] so every matmul is weight-stationary
(lhsT = W^T tiles, rhs = activation) with no runtime transposes.

Math restructurings (all exact):
  - sigmoid(z) = (1 + tanh(z/2))/2 everywhere -> the whole kernel only ever needs
    Exp and Tanh, which live in one ACT table set (no table-switch cost).
    The LSTM "g" gate's weights/bias are pre-doubled so one Tanh(scale=0.5) op
    covers all four gates; h and c are carried as 2h ("Hs") and 2c ("S").
  - gamma_h is pre-halved (bias ln(0.5) folded into the exp) so h' = (2h)*(gh/2).
  - x_c / c_c blends are expanded so the per-step output is
        out_t = c_c_t = S1_t + A_t*(Wf@q_t) + B_t*x_h_t
    with per-step streams precomputed on-device:
        S1 = m*x + A*(Wf@(m*x) + bfeat),  A = (1-m)*alpha,  B = (1-m) - A
        q  = 0.5*(1-m)*x_h  (Wf is pre-doubled to compensate)
"""

import os
from contextlib import ExitStack

import numpy as np

import concourse.bass as bass
import concourse.bacc as bacc
import concourse.tile as tile
from concourse import mybir

F32 = mybir.dt.float32
F16 = mybir.dt.float16
AF = mybir.ActivationFunctionType
ALU = mybir.AluOpType

# Problem dims
T_FULL, F_DIM, H_DIM, B_FULL = 256, 128, 256, 256
NCORES = 8
BC = 64              # batch per core
PC = 8               # precompute chunk (time steps); 8*64 = 512 free columns
CK = 16              # recurrence stream chunk (time steps)

LN_HALF = float(np.log(0.5))


def build_program(T=T_FULL):
    nc = bacc.Bacc(None, target_bir_lowering=False, debug=False)

    P = 128
    NJ = 8  # gate M-chunks (4H/128)

    # ---------------- DRAM I/O ----------------
    xT_d = nc.dram_tensor("xT", [P, T, BC], F32, kind="ExternalInput").ap()
    mT16_d = nc.dram_tensor("mT16", [P, T, BC], F16, kind="ExternalInput").ap()
    dT16_d = nc.dram_tensor("dT16", [P, T, BC], F16, kind="ExternalInput").ap()

    WdhT_d = nc.dram_tensor("WdhT", [P, 2, P], F16, kind="ExternalInput").ap()
    nbdh_d = nc.dram_tensor("nbdh", [P, 2], F32, kind="ExternalInput").ap()
    ddx_d = nc.dram_tensor("ddx", [P, 1], F32, kind="ExternalInput").ap()
    nbdx_d = nc.dram_tensor("nbdx", [P, 1], F32, kind="ExternalInput").ap()
    WcombT_d = nc.dram_tensor("WcombT", [P, 2, P], F16, kind="ExternalInput").ap()
    bcombh_d = nc.dram_tensor("bcombh", [P, 1], F32, kind="ExternalInput").ap()
    WhistT_d = nc.dram_tensor("WhistT", [P, 2, P], F16, kind="ExternalInput").ap()
    bhist_d = nc.dram_tensor("bhist", [P, 1], F32, kind="ExternalInput").ap()
    Wf2T_d = nc.dram_tensor("Wf2T", [P, P], F16, kind="ExternalInput").ap()
    bfeat_d = nc.dram_tensor("bfeat", [P, 1], F32, kind="ExternalInput").ap()
    WgT_d = nc.dram_tensor("WgT", [P, 4, 4 * H_DIM], F16, kind="ExternalInput").ap()
    bias16_d = nc.dram_tensor("bias16", [1, NJ, P], F16, kind="ExternalInput").ap()

    outT_d = nc.dram_tensor("outT", [P, T, BC], F32, kind="ExternalOutput").ap()

    # Device-internal streams
    gh_d = nc.dram_tensor("gh_s", [P, T, 2, BC], F16).ap()
    ommh_d = nc.dram_tensor("ommh_s", [P, T, BC], F16).ap()
    A_d = nc.dram_tensor("A_s", [P, T, BC], F16).ap()
    B_d = nc.dram_tensor("B_s", [P, T, BC], F16).ap()
    S1_d = nc.dram_tensor("S1_s", [P, T, BC], F32).ap()
    S116_d = nc.dram_tensor("S116_s", [P, T, BC], F16).ap()

    n_pre = T // PC
    n_rec = T // CK

    with tile.TileContext(nc) as tc, ExitStack() as ctx:
        wpool = ctx.enter_context(tc.tile_pool(name="weights", bufs=1))
        pre_in = ctx.enter_context(tc.tile_pool(name="pre_in", bufs=3))
        pre_sc = ctx.enter_context(tc.tile_pool(name="pre_sc", bufs=2))
        pre_ps = ctx.enter_context(tc.tile_pool(name="pre_ps", bufs=2, space="PSUM"))
        rec_st = ctx.enter_context(tc.tile_pool(name="rec_streams", bufs=2))
        rec_sc = ctx.enter_context(tc.tile_pool(name="rec_scratch", bufs=3))
        state_p = ctx.enter_context(tc.tile_pool(name="state", bufs=3))
        rec_ps = ctx.enter_context(tc.tile_pool(name="rec_ps", bufs=2, space="PSUM"))
        rec_psg = ctx.enter_context(tc.tile_pool(name="rec_psg", bufs=2, space="PSUM"))

        # ---------------- load weights ----------------
        WdhT = wpool.tile([P, 2, P], F16)
        nbdh = wpool.tile([P, 2], F32)
        ddx = wpool.tile([P, 1], F32)
        nbdx = wpool.tile([P, 1], F32)
        WcombT = wpool.tile([P, 2, P], F16)
        bcombh = wpool.tile([P, 1], F32)
        WhistT = wpool.tile([P, 2, P], F16)
        bhist = wpool.tile([P, 1], F32)
        Wf2T = wpool.tile([P, P], F16)
        bfeat = wpool.tile([P, 1], F32)
        WgT = wpool.tile([P, 4, 4 * H_DIM], F16)
        bias16 = wpool.tile([1, NJ, P], F16)
        ones16 = wpool.tile([1, BC], F16)
        nc.vector.memset(ones16[:], 1.0)
        for sb, dr in [(WdhT, WdhT_d), (nbdh, nbdh_d), (ddx, ddx_d), (nbdx, nbdx_d),
                       (WcombT, WcombT_d), (bcombh, bcombh_d), (WhistT, WhistT_d),
                       (bhist, bhist_d), (Wf2T, Wf2T_d), (bfeat, bfeat_d),
                       (WgT, WgT_d), (bias16, bias16_d)]:
            nc.sync.dma_start(out=sb, in_=dr)

        # ---------------- precompute phase ----------------
        for pc in range(n_pre):
            t0 = pc * PC
            x_t = pre_in.tile([P, PC, BC], F32, tag="x_t")
            m16_t = pre_in.tile([P, PC, BC], F16, tag="m16_t")
            d16_t = pre_in.tile([P, PC, BC], F16, tag="d16_t")
            nc.sync.dma_start(out=x_t, in_=xT_d[:, t0:t0 + PC, :])
            nc.sync.dma_start(out=m16_t, in_=mT16_d[:, t0:t0 + PC, :])
            nc.sync.dma_start(out=d16_t, in_=dT16_d[:, t0:t0 + PC, :])

            # gamma_h (pre-halved): gh = min(0.5*exp(-(Wdh@d + bdh)), 0.5)
            ghst = pre_sc.tile([P, PC, 2, BC], F16, tag="ghst")
            for jh in range(2):
                psg = pre_ps.tile([P, PC, BC], F32, tag="pre_mm")
                nc.tensor.matmul(psg[:], WdhT[:, jh, :], d16_t[:], start=True, stop=True)
                e = pre_sc.tile([P, PC, BC], F32, tag="gh_e")
                nc.scalar.activation(e[:], psg[:], AF.Exp, bias=nbdh[:, jh:jh + 1], scale=-1.0)
                nc.vector.tensor_scalar_min(ghst[:, :, jh, :], e[:], 0.5)
            nc.sync.dma_start(out=gh_d[:, t0:t0 + PC, :, :], in_=ghst[:])

            # gamma_x: gx = min(exp(-(ddx*d + bdx)), 1)
            y = pre_sc.tile([P, PC, BC], F32, tag="gx_y")
            nc.vector.tensor_scalar(y[:], d16_t[:], ddx[:], None, ALU.mult)
            ex = pre_sc.tile([P, PC, BC], F32, tag="gx_e")
            nc.scalar.activation(ex[:], y[:], AF.Exp, bias=nbdx[:], scale=-1.0)
            gx16 = pre_sc.tile([P, PC, BC], F16, tag="gx16")
            nc.vector.tensor_scalar_min(gx16[:], ex[:], 1.0)

            # alpha via tanh: th_a = tanh((Wcomb@[gx;m] + bcomb)/2); alpha=(1+th_a)/2
            psa = pre_ps.tile([P, PC, BC], F32, tag="pre_mm")
            nc.tensor.matmul(psa[:], WcombT[:, 0, :], gx16[:], start=True, stop=False)
            nc.tensor.matmul(psa[:], WcombT[:, 1, :], m16_t[:], start=False, stop=True)
            tha = pre_sc.tile([P, PC, BC], F32, tag="tha")
            nc.scalar.activation(tha[:], psa[:], AF.Tanh, bias=bcombh[:], scale=0.5)

            # ommh = 0.5*(1-m); A = (1+th_a)*ommh = (1-m)*alpha; B = 2*ommh - A
            ommh = pre_sc.tile([P, PC, BC], F16, tag="ommh")
            nc.vector.tensor_scalar(ommh[:], m16_t[:], -0.5, 0.5, ALU.mult, ALU.add)
            A_t = pre_sc.tile([P, PC, BC], F16, tag="A_t")
            nc.vector.scalar_tensor_tensor(A_t[:], tha[:], 1.0, ommh[:], ALU.add, ALU.mult)
            # gpsimd has no scalar_tensor_tensor (walrus engine check): 2 ops
            om2 = pre_sc.tile([P, PC, BC], F32, tag="om2")
            nc.gpsimd.tensor_scalar_mul(om2[:], ommh[:], 2.0)
            B_t = pre_sc.tile([P, PC, BC], F16, tag="B_t")
            nc.gpsimd.tensor_tensor(B_t[:], om2[:], A_t[:], ALU.subtract)
            nc.sync.dma_start(out=ommh_d[:, t0:t0 + PC, :], in_=ommh[:])
            nc.sync.dma_start(out=A_d[:, t0:t0 + PC, :], in_=A_t[:])
            nc.sync.dma_start(out=B_d[:, t0:t0 + PC, :], in_=B_t[:])

            # mx = m*x ; S1 = mx + A*(Wf@mx + bfeat)
            mx = pre_sc.tile([P, PC, BC], F32, tag="mx")
            nc.gpsimd.tensor_tensor(mx[:], m16_t[:], x_t[:], ALU.mult)
            mxh16 = pre_sc.tile([P, PC, BC], F16, tag="mxh16")
            nc.gpsimd.tensor_scalar_mul(mxh16[:], mx[:], 0.5)
            psz = pre_ps.tile([P, PC, BC], F32, tag="pre_mm")
            nc.tensor.matmul(psz[:], Wf2T[:], mxh16[:], start=True, stop=True)
            azx = pre_sc.tile([P, PC, BC], F32, tag="azx")
            nc.vector.scalar_tensor_tensor(azx[:], psz[:], bfeat[:], A_t[:], ALU.add, ALU.mult)
            S1_t = pre_sc.tile([P, PC, BC], F32, tag="S1_t")
            nc.vector.tensor_tensor(S1_t[:], mx[:], azx[:], ALU.add)
            nc.sync.dma_start(out=S1_d[:, t0:t0 + PC, :], in_=S1_t[:])
            S116_t = pre_sc.tile([P, PC, BC], F16, tag="S116_t")
            nc.gpsimd.tensor_copy(out=S116_t[:], in_=S1_t[:])
            nc.sync.dma_start(out=S116_d[:, t0:t0 + PC, :], in_=S116_t[:])

        # ---------------- recurrence phase (single chain, B=64) ----------------
        # state: Hs = 2h [P, 2, BC], Sc = 2c [P, 2, BC]
        Hs = state_p.tile([P, 2, BC], F32, tag="Hs", name="Hs0")
        Sc = state_p.tile([P, 2, BC], F32, tag="Sc", name="Sc0")
        nc.vector.memset(Hs[:], 0.0)
        nc.vector.memset(Sc[:], 0.0)

        hp16 = rec_sc.tile([P, 2, BC], F16, tag="hp16", name="hp16_init")
        nc.vector.memset(hp16[:], 0.0)   # h' at t=0 is 0 (h0 = 0)

        for rc in range(n_rec):
            t0 = rc * CK
            ghc = rec_st.tile([P, CK, 2, BC], F16, tag="ghc")
            ommc = rec_st.tile([P, CK, BC], F16, tag="ommc")
            Ac = rec_st.tile([P, CK, BC], F16, tag="Ac")
            Bc = rec_st.tile([P, CK, BC], F16, tag="Bc")
            S1c = rec_st.tile([P, CK, BC], F32, tag="S1c")
            S116c = rec_st.tile([P, CK, BC], F16, tag="S116c")
            mc16 = rec_st.tile([P, CK, BC], F16, tag="mc16")
            outc = rec_st.tile([P, CK, BC], F32, tag="outc")
            nc.sync.dma_start(out=ghc, in_=gh_d[:, t0:t0 + CK, :, :])
            nc.sync.dma_start(out=ommc, in_=ommh_d[:, t0:t0 + CK, :])
            nc.sync.dma_start(out=Ac, in_=A_d[:, t0:t0 + CK, :])
            nc.sync.dma_start(out=Bc, in_=B_d[:, t0:t0 + CK, :])
            nc.sync.dma_start(out=S1c, in_=S1_d[:, t0:t0 + CK, :])
            nc.sync.dma_start(out=S116c, in_=S116_d[:, t0:t0 + CK, :])
            nc.sync.dma_start(out=mc16, in_=mT16_d[:, t0:t0 + CK, :])

            for tl in range(CK):
                t_glob = t0 + tl
                if rc > 0 and tl == 0:
                    # rebuild h' from the boundary H with this chunk's gh
                    hp16 = rec_sc.tile([P, 2, BC], F16, tag="hp16", name=f"hp16_b{rc}")
                    nc.vector.tensor_tensor(hp16[:], Hs[:], ghc[:, 0, :, :], ALU.mult)
                # gates psum group; bias rows first (input-independent, the
                # j==0 bias MM start zeroes the whole bank/zero-region).
                psg = rec_psg.tile([P, NJ, BC], F32, tag="psg")
                for j in range(NJ):
                    nc.tensor.matmul(psg[:, j, :], bias16[:, j, :], ones16[:],
                                     start=(j == 0), stop=False)
                # stream-ready parts: S1 (cc-part), m — can run before hp16
                for kc, rhs in [(0, S116c[:, tl, :]), (1, mc16[:, tl, :])]:
                    for j in range(NJ):
                        nc.tensor.matmul(psg[:, j, :], WgT[:, kc, j * P:(j + 1) * P],
                                         rhs, start=False, stop=False)

                # x_h path first (spine): psxh = Whist@h'
                psxh = rec_ps.tile([P, BC], F32, tag="psxh")
                nc.tensor.matmul(psxh[:], WhistT[:, 0, :], hp16[:, 0, :], start=True, stop=False)
                nc.tensor.matmul(psxh[:], WhistT[:, 1, :], hp16[:, 1, :], start=False, stop=True)

                # h-parts of the gates (off-spine, fill PE while DVE works)
                for kc, rhs in [(2, hp16[:, 0, :]), (3, hp16[:, 1, :])]:
                    for j in range(NJ):
                        nc.tensor.matmul(psg[:, j, :], WgT[:, kc, j * P:(j + 1) * P],
                                         rhs, start=False, stop=False)

                # q16 = ommh*x_h, a2 = B*x_h   [x_h = psxh + bhist]
                q16 = rec_sc.tile([P, BC], F16, tag="q16")
                nc.vector.scalar_tensor_tensor(q16[:], psxh[:], bhist[:], ommc[:, tl, :], ALU.add, ALU.mult)
                a2 = rec_sc.tile([P, BC], F16, tag="a2")
                nc.vector.scalar_tensor_tensor(a2[:], psxh[:], bhist[:], Bc[:, tl, :], ALU.add, ALU.mult)

                # z path: psz = (2Wf)@q = Wf@((1-m)*x_h); a1 = A*psz
                psz = rec_ps.tile([P, BC], F32, tag="psz")
                nc.tensor.matmul(psz[:], Wf2T[:], q16[:], start=True, stop=True)
                a1 = rec_sc.tile([P, BC], F16, tag="a1")
                nc.vector.tensor_tensor(a1[:], psz[:], Ac[:, tl, :], ALU.mult)

                # gates: cc-part decomposed as Wcc@(S1 + a1 + a2); S1 done above,
                # a1/a2 here (kc=0 lhsT reused back-to-back per j)
                for j in range(NJ):
                    nc.tensor.matmul(psg[:, j, :], WgT[:, 0, j * P:(j + 1) * P],
                                     a1[:], start=False, stop=False)
                    nc.tensor.matmul(psg[:, j, :], WgT[:, 0, j * P:(j + 1) * P],
                                     a2[:], start=False, stop=(j == NJ - 1))

                # out_t = c_c = S1 + a1 + a2 (off critical path, on Pool)
                v = rec_sc.tile([P, BC], F32, tag="v")
                nc.gpsimd.tensor_tensor(v[:], a1[:], a2[:], ALU.add)
                nc.gpsimd.tensor_tensor(outc[:, tl, :], v[:], S1c[:, tl, :], ALU.add)

                # tanh(z/2) from PSUM (bias pre-accumulated); order (i,g | f,o)
                th = rec_sc.tile([P, NJ, BC], F32, tag="th")
                nc.scalar.activation(th[:, 0:4, :], psg[:, 0:4, :], AF.Tanh, scale=0.5)
                nc.scalar.activation(th[:, 4:8, :], psg[:, 4:8, :], AF.Tanh, scale=0.5)
                m2 = rec_sc.tile([P, 2, BC], F32, tag="m2")
                nc.vector.scalar_tensor_tensor(m2[:], th[:, 0:2, :], 1.0, th[:, 2:4, :], ALU.add, ALU.mult)
                m1 = rec_sc.tile([P, 2, BC], F32, tag="m1")
                nc.vector.scalar_tensor_tensor(m1[:], th[:, 4:6, :], 1.0, Sc[:], ALU.add, ALU.mult)
                S_new = state_p.tile([P, 2, BC], F32, tag="Sc")
                nc.vector.scalar_tensor_tensor(S_new[:], m1[:], 0.5, m2[:], ALU.mult, ALU.add)
                Sc = S_new

                # tc = tanh(c'); next h' = 0.5*(1+th_o)*tc*gh_{t+1} fused
                tc_t = rec_sc.tile([P, 2, BC], F32, tag="tc_t")
                nc.scalar.activation(tc_t[:], S_new[:], AF.Tanh, scale=0.5)
                if t_glob == T - 1:
                    break
                if tl < CK - 1:
                    w = rec_sc.tile([P, 2, BC], F32, tag="w")
                    nc.vector.tensor_tensor(w[:], tc_t[:], ghc[:, tl + 1, :, :], ALU.mult)
                    hp16 = rec_sc.tile([P, 2, BC], F16, tag="hp16")
                    nc.vector.scalar_tensor_tensor(hp16[:], th[:, 6:8, :], 1.0, w[:], ALU.add, ALU.mult)
                else:
                    # chunk boundary: materialize H = (1+th_o)*tc; next chunk's
                    # first step builds hp16 from it.
                    H_new = state_p.tile([P, 2, BC], F32, tag="Hs")
                    nc.vector.scalar_tensor_tensor(H_new[:], th[:, 6:8, :], 1.0, tc_t[:], ALU.add, ALU.mult)
                    Hs = H_new

            nc.sync.dma_start(out=outT_d[:, t0:t0 + CK, :], in_=outc[:])

    nc.compile()
    return nc


# ---------------------------------------------------------------------------
# Host-side preparation
# ---------------------------------------------------------------------------

def _prep_params(p):
    """Per-direction parameter pack -> device tensors (numpy)."""
    P = 128
    f32 = np.float32
    f16 = np.float16
    Wdh = np.asarray(p['Wdh'], f32)       # [H, F]
    bdh = np.asarray(p['bdh'], f32)       # [H]
    Wdx = np.asarray(p['Wdx'], f32)       # [F, F]
    bdx = np.asarray(p['bdx'], f32)
    Whist = np.asarray(p['Whist'], f32)   # [F, H]
    bhist = np.asarray(p['bhist'], f32)
    Wfeat = np.asarray(p['Wfeat'], f32)   # [F, F]
    bfeat = np.asarray(p['bfeat'], f32)
    Wcomb = np.asarray(p['Wcomb'], f32)   # [F, 2F]
    bcomb = np.asarray(p['bcomb'], f32)
    Wih = np.asarray(p['Wih'], f32)       # [4H, 2F]
    bih = np.asarray(p['bih'], f32)
    Whh = np.asarray(p['Whh'], f32)       # [4H, H]
    bhh = np.asarray(p['bhh'], f32)

    H = H_DIM
    out = {}
    out['WdhT'] = np.ascontiguousarray(Wdh.T.reshape(P, 2, P, order='F')
                                       if False else Wdh.T.reshape(P, 2, P)).astype(f16)
    # Wdh.T is [F=128, H=256]; chunk halves along H: [128, 2, 128]
    out['WdhT'] = np.ascontiguousarray(Wdh.T.reshape(P, 2, P)).astype(f16)
    out['nbdh'] = np.ascontiguousarray((-bdh + LN_HALF).reshape(2, P).T).astype(f32)
    out['ddx'] = np.ascontiguousarray(np.diag(Wdx).reshape(P, 1)).astype(f32)
    out['nbdx'] = np.ascontiguousarray(-bdx.reshape(P, 1)).astype(f32)
    out['WcombT'] = np.ascontiguousarray(Wcomb.T.reshape(2, P, P).transpose(1, 0, 2)).astype(f16)
    out['bcombh'] = np.ascontiguousarray(0.5 * bcomb.reshape(P, 1)).astype(f32)
    out['WhistT'] = np.ascontiguousarray(Whist.T.reshape(2, P, P).transpose(1, 0, 2)).astype(f16)
    out['bhist'] = np.ascontiguousarray(bhist.reshape(P, 1)).astype(f32)
    Wf = Wfeat * (1.0 - np.eye(F_DIM, dtype=f32))
    out['Wf2T'] = np.ascontiguousarray((2.0 * Wf).T).astype(f16)
    out['bfeat'] = np.ascontiguousarray(bfeat.reshape(P, 1)).astype(f32)

    # gates: reorder torch (i,f,g,o) -> (i,g,f,o); double g rows (tanh trick);
    # combine [Wih|Whh]
    perm = np.concatenate([np.arange(0, H), np.arange(2 * H, 3 * H),
                           np.arange(H, 2 * H), np.arange(3 * H, 4 * H)])
    W_all = np.concatenate([Wih, Whh], axis=1)[perm]          # [1024, 512]
    b_all = (bih + bhh)[perm].copy()
    W_all[H:2 * H] *= 2.0
    b_all[H:2 * H] *= 2.0
    # lhsT = W_all.T [512, 1024] -> [4 k-chunks, 128, 1024] -> [128, 4, 1024]
    out['WgT'] = np.ascontiguousarray(W_all.T.reshape(4, P, 4 * H).transpose(1, 0, 2)).astype(f16)
    # bias16 [1, 8, 128]: bias row chunks, injected via K=1 ones-matmuls
    out['bias16'] = np.ascontiguousarray(b_all.reshape(1, 8, P)).astype(f16)
    return out


def _prep_data(X_s, M_s, D_s):
    """[Bc, T, F] float arrays -> transposed device streams [F, T, Bc]."""
    xT = np.ascontiguousarray(np.transpose(X_s, (2, 1, 0))).astype(np.float32)
    mT16 = np.ascontiguousarray(np.transpose(M_s, (2, 1, 0))).astype(np.float16)
    dT16 = np.ascontiguousarray(np.transpose(D_s, (2, 1, 0))).astype(np.float16)
    return {'xT': xT, 'mT16': mT16, 'dT16': dT16}


_NC_CACHE = {}


def _get_program(T=T_FULL):
    if T not in _NC_CACHE:
        _NC_CACHE[T] = build_program(T)
    return _NC_CACHE[T]


def make_in_maps(X, missing_mask, deltas_f, deltas_b, params_f, params_b, T=T_FULL):
    X = np.asarray(X, np.float32)
    M = np.asarray(missing_mask, np.float32)
    Df = np.asarray(deltas_f, np.float32)
    Db = np.asarray(deltas_b, np.float32)
    pf = _prep_params(params_f)
    pb = _prep_params(params_b)
    in_maps = []
    for c in range(NCORES):
        q = c % 4
        bs = slice(q * BC, (q + 1) * BC)
        if c < 4:
            data = _prep_data(X[bs, :T], M[bs, :T], Df[bs, :T])
            im = dict(data, **pf)
        else:
            data = _prep_data(X[bs, :T][:, ::-1], M[bs, :T][:, ::-1], Db[bs, :T])
            im = dict(data, **pb)
        in_maps.append(im)
    return in_maps


def gather_output(results, T=T_FULL):
    """results: list of 8 dicts with 'outT' [128, T, BC] -> full [B, T, F] avg."""
    out = np.empty((B_FULL, T, F_DIM), np.float32)
    for q in range(4):
        f = results[q]["outT"].reshape(F_DIM, T, BC).transpose(2, 1, 0)
        b = results[4 + q]["outT"].reshape(F_DIM, T, BC).transpose(2, 1, 0)[:, ::-1]
        out[q * BC:(q + 1) * BC] = 0.5 * (f + b)
    return out


def kernel(X, missing_mask, deltas_f, deltas_b, params_f, params_b):
    from concourse.bass_utils import run_bass_kernel_spmd
    nc = _get_program()
    in_maps = make_in_maps(X, missing_mask, deltas_f, deltas_b, params_f, params_b)
    res = run_bass_kernel_spmd(nc, in_maps, list(range(NCORES)))
    return gather_output(res.results)
